# revision 55
# baseline (speedup 1.0000x reference)
"""NT-Xent / InfoNCE contrastive loss (SimCLR) on 8 TRN2 NeuronCores.

Problem: features [8192, 1024] f32.
  f = features / ||features||_row ; sim = f @ f.T / 0.07
  pos_i = sim[i, (i+4096) mod 8192] ; denom_i = logsumexp_{j!=i} sim[i,j]
  loss = mean(denom - pos)

v3 design — symmetric-triangle + fp8 DoubleRow:

Sharding: core k's input is rolled by -1024k rows, so its local rows are
row-tiles 0..7 (128 rows each) of its own view. sim is symmetric, so each
128x128 block pair {i,j} is computed once globally: core-of-tile-t computes
blocks (m, m+d) for local tile m=0..7 and offset d=0..32 (264 blocks/core
vs 512 for the full strip). d=0 is the self block (diag masked, row-sums
only); d in 1..31 contributes row-sums for tile m AND column-sums for tile
m+d (the mirrored block's row-sums, by symmetry exp(B)^T = exp(B^T)); d=32
contributes row-sums only (the mirror core computes its own side) and its
diagonal is the positive pair.

Device pipeline per column-group g (software-pipelined A(g+2)/R(g)/B(g),
with main chunks of groups <= g interleaved so PE/ACT fill during prep):
  A. HWDGE loads of the host-pretransposed bf16 input xT -> fT[ks][g];
     DVE squares; sums-of-squares via near-free PE matmuls (sq-subtile
     stationary x ones -> psum[p,1] = per-row sumsq, k-accumulated
     ks-inner since PSUM accumulation groups are bank-granular);
     ACT rsqrt as exp(-0.5 ln(ss/SCALE^2))
  R. SWDGE transposed write of the scales to DRAM (rt[128t+p]=rbf[p,t])
     and stride-0 partition-broadcast read back -> R [128, W]
  B. DVE normalize fT*R -> bf16 nstage; one SWDGE cast-DMA -> fp8 fqn
Main phase per (m, g) group-aligned chunk: fp8e4m3 DoubleRow matmuls
(0.5 cyc/row, 2 k-slices per instruction via [128,2,N] APs) accumulate
logits B in PSUM over 4 k-pairs; diag mask for d=0 applied as one extra
bf16 matmul (-1e9*I = negI^T @ I) inside the accumulation group; ACT exp
with free-axis accum gives row-sum partials; column-sums of each 128-col
subtile via near-free matmuls (E-subtile stationary x ones -> [128,1],
colsums landed on partitions); positive pair = diag of the exp'd d=32
tile via DVE eye-mask multiply + reduce, ln taken on host.

Host: combines row/col partial sums across cores (rolled back), takes ln,
subtracts ln(exp-pos), means. All O(N^2) work stays on device.
"""

import math
import sys

import numpy as np

try:  # concourse is normally on sys.path via the site config
    import concourse  # noqa: F401
except ImportError:  # pragma: no cover
    for _p in ("/opt/trn_rl_repo", "/root/.axon_site/_ro/trn_rl_repo"):
        if _p not in sys.path:
            sys.path.insert(0, _p)

N = 8192
D = 1024
P = 128
NCORES = 8
KS = 8  # 128-row k-slices of the contraction dim
NDOFF = 33  # block offsets d = 0..32 per local row tile
TCOL = 40  # column tiles needed: m + d <= 7 + 32
W = TCOL * P  # 5120 gathered columns materialized per core
GBOUNDS = [0, 8, 16, 24, 32, 40]  # column-group tile boundaries
NG = len(GBOUNDS) - 1  # small lead groups shorten the first cast's chain
GW = 1024  # widest group, used for buffer padding
TEMPERATURE = 0.07
INVT = 1.0 / TEMPERATURE
SCALE = 64.0  # fp8 operand scale: fq = SCALE * f_hat
S_EXP = INVT / (SCALE * SCALE)  # exp(S_EXP * B) == exp(sim / T)
DIAG_NEG = -1.0e9  # self-sim mask added to PSUM pre-exp

ACT_SET = "natural_log_exp_and_others"  # contains exp, ln

_cache = {}


def _build_program():
    import concourse.bacc as bacc
    import concourse.mybir as mybir
    from concourse import tile

    f32 = mybir.dt.float32
    bf16 = mybir.dt.bfloat16
    fp8 = mybir.dt.float8e4
    AF = mybir.ActivationFunctionType
    AX = mybir.AxisListType
    ALU = mybir.AluOpType
    DR = mybir.MatmulPerfMode.DoubleRow

    # Pin every activation to one LUT set so the table-load pass emits a
    # single load instead of thrashing between per-function default sets.
    orig_tables = bacc.get_activation_tables

    def pinned_tables(arch):
        return {
            name: (funcs if name == ACT_SET else set())
            for name, funcs in orig_tables(arch).items()
        }

    bacc.get_activation_tables = pinned_tables
    try:
        nc = bacc.Bacc(
            "TRN2",
            target_bir_lowering=False,
            debug=False,
            num_devices=NCORES,
        )
        xT = nc.declare_dram_parameter("xT", [D, W], bf16, isOutput=False)
        eye_d = nc.declare_dram_parameter("eye", [P, P], bf16, isOutput=False)
        negI_d = nc.declare_dram_parameter("negI", [P, P], bf16, isOutput=False)
        # single packed output: [rs (8*NG) | cs (248) | epos (8)]
        out_d = nc.declare_dram_parameter(
            "out", [P, 8 * NG + 8 * 31 + 8], f32, isOutput=True
        )
        rt_dram = nc.dram_tensor("rt_dram", [W], bf16)

        with tile.TileContext(nc) as tc:
            with (
                tc.tile_pool(name="big", bufs=1) as big,
                tc.tile_pool(name="ftp", bufs=2) as ftp,
                tc.tile_pool(name="work", bufs=3) as work,
                tc.tile_pool(name="nst", bufs=2) as nst,
                tc.tile_pool(name="psB", bufs=2, space="PSUM") as psB,
                tc.tile_pool(name="psS", bufs=1, space="PSUM") as psS,
            ):
                eye_sb = big.tile([P, P], bf16, tag="eye")
                negI_sb = big.tile([P, P], bf16, tag="negI")
                ones_sb = big.tile([P, 1], bf16, tag="ones")
                nc.vector.memset(ones_sb[:], 1.0)

                fqn = big.tile([P, KS, W], fp8, tag="fqn")
                R = big.tile([P, W], bf16, tag="R")
                rbf = big.tile([P, TCOL], bf16, tag="rbf")
                out_sb = big.tile([P, 8 * NG + 8 * 31 + 8], f32, tag="osb")
                rs_sb = out_sb[:, 0 : 8 * NG]
                cs_sb = out_sb[:, 8 * NG : 8 * NG + 8 * 31]
                epos_sb = out_sb[:, 8 * NG + 8 * 31 : 8 * NG + 8 * 31 + 8]

                # PSUM accumulation groups are tracked at 2KB-bank (zero
                # region) granularity: pad both tiles to a full bank so no
                # two concurrently-open groups ever share a bank.
                ss = psS.tile(
                    [P, TCOL], f32, tag="ss", name="ss", padded_shape=[P, 512]
                )
                cs_ps = psS.tile(
                    [P, 8 * 31], f32, tag="csps", name="csps", padded_shape=[P, 512]
                )

                fts = {}
                nstages = {}

                def prep_load(g):
                    tb0, tb1 = GBOUNDS[g], GBOUNDS[g + 1]
                    c0, w = tb0 * P, (tb1 - tb0) * P
                    for ks in range(KS):
                        ft = ftp.tile(
                            [P, w],
                            bf16,
                            tag=f"fT{ks}",
                            name=f"fT{ks}",
                            bufs=3,
                            padded_shape=[P, GW],
                        )
                        fts[(ks, g)] = ft
                        nc.sync.dma_start(
                            ft[:], xT[ks * P : (ks + 1) * P, c0 : c0 + w]
                        )

                def prep_a(g):
                    tb0, tb1 = GBOUNDS[g], GBOUNDS[g + 1]
                    nt = tb1 - tb0
                    c0, w = tb0 * P, nt * P
                    # All of ss lives in one PSUM bank, and group state is
                    # bank-granular — so run each column's 8-matmul k-accum
                    # group to completion (ks-inner) before opening the next.
                    sqs = {}
                    for ks in range(KS):
                        sq = sqs[ks] = work.tile(
                            [P, w],
                            bf16,
                            tag=f"sq{ks}",
                            name="sq",
                            bufs=2,
                            padded_shape=[P, GW],
                        )
                        nc.vector.tensor_mul(sq[:], fts[(ks, g)][:], fts[(ks, g)][:])
                    for s in range(nt):
                        t = tb0 + s
                        for ks in range(KS):
                            nc.tensor.matmul(
                                ss[:, t : t + 1],
                                sqs[ks][:, s * P : (s + 1) * P],
                                ones_sb[:],
                                start=(ks == 0),
                                stop=(ks == KS - 1),
                            )
                    # rsqrt with the fp8 scale folded in: SCALE * ss^-0.5
                    lnss = work.tile(
                        [P, nt], f32, tag="lnss", name="lnss", padded_shape=[P, 8]
                    )
                    nc.scalar.activation(
                        lnss[:],
                        ss[:, tb0:tb1],
                        AF.Ln,
                        scale=1.0 / (SCALE * SCALE),
                    )
                    nc.scalar.activation(
                        rbf[:, tb0:tb1], lnss[:], AF.Exp, scale=-0.5
                    )

                def prep_r(g):
                    tb0, tb1 = GBOUNDS[g], GBOUNDS[g + 1]
                    c0, w = tb0 * P, (tb1 - tb0) * P
                    # transposed write to DRAM (rt[128t+p] = rbf[p,t]; DRAM
                    # strides are unconstrained), then stride-0 partition
                    # broadcast-read back into all 128 partitions of R.
                    # On the SWDGE ring: HWDGE rings would head-of-line block
                    # the issuing engine's SEQ (R waits on rt's completion).
                    nc.gpsimd.dma_start(
                        rt_dram[c0 : c0 + w].rearrange("(t p) -> p t", p=P),
                        rbf[:, tb0:tb1],
                    )
                    nc.gpsimd.dma_start(
                        R[:, c0 : c0 + w],
                        rt_dram[c0 : c0 + w]
                        .rearrange("(a x) -> a x", a=1)
                        .broadcast_to((P, w)),
                    )

                def prep_b(g):
                    tb0, tb1 = GBOUNDS[g], GBOUNDS[g + 1]
                    c0, w = tb0 * P, (tb1 - tb0) * P
                    nstage = nst.tile(
                        [P, KS, w],
                        bf16,
                        tag="nst",
                        name="nst",
                        padded_shape=[P, KS, GW],
                    )
                    nstages[g] = nstage
                    for ks in range(KS):
                        nc.vector.tensor_mul(
                            nstage[:, ks, :], fts[(ks, g)][:], R[:, c0 : c0 + w]
                        )
                    nc.gpsimd.dma_start(fqn[:, :, c0 : c0 + w], nstage[:, :, :])

                def main_chunk(m, g):
                    # group-aligned chunk: intersection of row-tile m's
                    # offset strip (tiles m..m+32) with column group g —
                    # ready as soon as cast(g) lands
                    t0 = max(GBOUNDS[g], m)
                    t1 = min(GBOUNDS[g + 1], m + NDOFF)
                    if t1 <= t0:
                        return
                    wch = (t1 - t0) * P
                    col0 = t0 * P
                    B = psB.tile(
                        [P, wch], f32, tag="B", name="B", padded_shape=[P, GW]
                    )
                    nh = (wch + 511) // 512
                    for j in range(4):
                        for h in range(nh):
                            h0 = h * 512
                            h1 = min(wch, h0 + 512)
                            nc.tensor.matmul(
                                B[:, h0:h1],
                                fqn[:, 2 * j : 2 * j + 2, m * P : (m + 1) * P],
                                fqn[:, 2 * j : 2 * j + 2, col0 + h0 : col0 + h1],
                                start=(j == 0),
                                stop=(j == 3),
                                perf_mode=DR,
                            )
                        if j == 2 and t0 == m:
                            # self-block diag mask: B[:, :128] += negI^T @ I
                            nc.tensor.matmul(
                                B[:, 0:P],
                                negI_sb[:],
                                eye_sb[:],
                                start=False,
                                stop=False,
                            )
                    E = work.tile(
                        [P, wch],
                        bf16,
                        tag="E",
                        name="E",
                        bufs=6,
                        padded_shape=[P, GW],
                    )
                    nc.scalar.activation(
                        E[:],
                        B[:],
                        AF.Exp,
                        scale=S_EXP,
                        accum_out=rs_sb[:, NG * m + g : NG * m + g + 1],
                    )
                    for t in range(t0, t1):
                        d = t - m
                        if d == 0 or d == 32:
                            continue  # self block / d=32: row side only
                        s0 = (t - t0) * P
                        nc.tensor.matmul(
                            cs_ps[:, m * 31 + d - 1 : m * 31 + d],
                            E[:, s0 : s0 + P],
                            ones_sb[:],
                            start=True,
                            stop=True,
                        )
                    if t0 <= m + 32 < t1:
                        # positive pair: diag of the exp'd d=32 tile
                        s0 = (m + 32 - t0) * P
                        pd = work.tile([P, P], bf16, tag="pd", name="pd")
                        nc.vector.tensor_mul(pd[:], E[:, s0 : s0 + P], eye_sb[:])
                        nc.vector.reduce_sum(
                            epos_sb[:, m : m + 1], pd[:], axis=AX.X
                        )

                # Emission order: A(g) = loads/squares/sumsq/rsqrt,
                # R(g) = scale transpose/broadcast, B(g) = normalize+cast.
                # A(g+1) is emitted before B(g) so DVE has square work while
                # R(g) settles; each cast(g) unlocks the 8 (m, g) chunks.
                prep_load(0)
                prep_a(0)
                # eye/negI aren't needed until the first diag-mask matmul
                # (~25us in): keep the SP ring clear for the g0/g1 loads
                nc.sync.dma_start(eye_sb[:], eye_d[:])
                nc.sync.dma_start(negI_sb[:], negI_d[:])
                prep_load(1)
                prep_a(1)
                prep_r(0)
                for g in range(NG):
                    # rt/R of g+1 ahead of cast(g) on the Pool queue: its
                    # rsqrt fired a DVE-period ago, so the hop completes
                    # while cast(g) still waits on norm(g)
                    if g + 1 < NG:
                        prep_r(g + 1)
                    prep_b(g)
                    if g + 2 < NG:
                        prep_load(g + 2)
                        prep_a(g + 2)
                    for m in range(8):
                        main_chunk(m, g)

                nc.vector.tensor_copy(cs_sb, cs_ps[:])
                nc.sync.dma_start(out_d[:], out_sb[:])

        nc.compile()
    finally:
        bacc.get_activation_tables = orig_tables
    return nc


def _get_program():
    if "nc" not in _cache:
        _cache["nc"] = _build_program()
    return _cache["nc"]


def kernel(features: np.ndarray, _trace: bool = False):
    import ml_dtypes
    from concourse.bass_utils import run_bass_kernel_spmd

    nc = _get_program()
    features = np.ascontiguousarray(features, dtype=np.float32)
    eye = np.eye(P, dtype=ml_dtypes.bfloat16)
    negI = (DIAG_NEG * np.eye(P)).astype(ml_dtypes.bfloat16)
    in_maps = []
    for k in range(NCORES):
        xk = np.roll(features, -N // NCORES * k, axis=0)
        xT = np.ascontiguousarray(xk.astype(ml_dtypes.bfloat16).T[:, :W])
        in_maps.append({"xT": xT, "eye": eye, "negI": negI})
    res = run_bass_kernel_spmd(
        nc,
        in_maps,
        core_ids=list(range(NCORES)),
        trace=_trace,
    )

    acc = np.zeros((64, P), np.float64)
    pos_ln = np.zeros((64, P), np.float64)
    for k in range(NCORES):
        o = res.results[k]["out"].astype(np.float64)
        rs = o[:, 0 : 8 * NG]
        cs = o[:, 8 * NG : 8 * NG + 248]
        epos = o[:, 8 * NG + 248 :]
        for m in range(8):
            t = (m + 8 * k) % 64
            acc[t] += rs[:, NG * m : NG * (m + 1)].sum(axis=1)
            pos_ln[t] = np.log(epos[:, m])
            for d in range(1, 32):
                acc[(m + d + 8 * k) % 64] += cs[:, m * 31 + d - 1]
    losses = np.log(acc) - pos_ln
    loss = np.float32(losses.mean())
    if _trace:
        return loss, res
    return loss


# revision 56
# speedup vs baseline: 1.0130x; 1.0130x over previous
"""NT-Xent / InfoNCE contrastive loss (SimCLR) on 8 TRN2 NeuronCores.

Problem: features [8192, 1024] f32.
  f = features / ||features||_row ; sim = f @ f.T / 0.07
  pos_i = sim[i, (i+4096) mod 8192] ; denom_i = logsumexp_{j!=i} sim[i,j]
  loss = mean(denom - pos)

v3 design — symmetric-triangle + fp8 DoubleRow:

Sharding: core k's input is rolled by -1024k rows, so its local rows are
row-tiles 0..7 (128 rows each) of its own view. sim is symmetric, so each
128x128 block pair {i,j} is computed once globally: core-of-tile-t computes
blocks (m, m+d) for local tile m=0..7 and offset d=0..32 (264 blocks/core
vs 512 for the full strip). d=0 is the self block (diag masked, row-sums
only); d in 1..31 contributes row-sums for tile m AND column-sums for tile
m+d (the mirrored block's row-sums, by symmetry exp(B)^T = exp(B^T)); d=32
contributes row-sums only (the mirror core computes its own side) and its
diagonal is the positive pair.

Device pipeline per column-group g (software-pipelined A(g+2)/R(g)/B(g),
with main chunks of groups <= g interleaved so PE/ACT fill during prep):
  A. HWDGE loads of the host-pretransposed bf16 input xT -> fT[ks][g];
     DVE squares; sums-of-squares via near-free PE matmuls (sq-subtile
     stationary x ones -> psum[p,1] = per-row sumsq, k-accumulated
     ks-inner since PSUM accumulation groups are bank-granular);
     ACT rsqrt as exp(-0.5 ln(ss/SCALE^2))
  R. SWDGE transposed write of the scales to DRAM (rt[128t+p]=rbf[p,t])
     and stride-0 partition-broadcast read back -> R [128, W]
  B. DVE normalize fT*R -> bf16 nstage; one SWDGE cast-DMA -> fp8 fqn
Main phase per (m, g) group-aligned chunk: fp8e4m3 DoubleRow matmuls
(0.5 cyc/row, 2 k-slices per instruction via [128,2,N] APs) accumulate
logits B in PSUM over 4 k-pairs; diag mask for d=0 applied as one extra
bf16 matmul (-1e9*I = negI^T @ I) inside the accumulation group; ACT exp
with free-axis accum gives row-sum partials; column-sums of each 128-col
subtile via near-free matmuls (E-subtile stationary x ones -> [128,1],
colsums landed on partitions); positive pair = diag of the exp'd d=32
tile via DVE eye-mask multiply + reduce, ln taken on host.

Host: combines row/col partial sums across cores (rolled back), takes ln,
subtracts ln(exp-pos), means. All O(N^2) work stays on device.
"""

import math
import sys

import numpy as np

try:  # concourse is normally on sys.path via the site config
    import concourse  # noqa: F401
except ImportError:  # pragma: no cover
    for _p in ("/opt/trn_rl_repo", "/root/.axon_site/_ro/trn_rl_repo"):
        if _p not in sys.path:
            sys.path.insert(0, _p)

N = 8192
D = 1024
P = 128
NCORES = 8
KS = 8  # 128-row k-slices of the contraction dim
NDOFF = 33  # block offsets d = 0..32 per local row tile
TCOL = 40  # column tiles needed: m + d <= 7 + 32
W = TCOL * P  # 5120 gathered columns materialized per core
GBOUNDS = [0, 8, 16, 24, 32, 40]  # column-group tile boundaries
NG = len(GBOUNDS) - 1  # small lead groups shorten the first cast's chain
GW = 1024  # widest group, used for buffer padding
TEMPERATURE = 0.07
INVT = 1.0 / TEMPERATURE
SCALE = 64.0  # fp8 operand scale: fq = SCALE * f_hat
S_EXP = INVT / (SCALE * SCALE)  # exp(S_EXP * B) == exp(sim / T)
DIAG_NEG = -1.0e9  # self-sim mask added to PSUM pre-exp

ACT_SET = "natural_log_exp_and_others"  # contains exp, ln

_cache = {}


def _build_program():
    import concourse.bacc as bacc
    import concourse.mybir as mybir
    from concourse import tile

    f32 = mybir.dt.float32
    bf16 = mybir.dt.bfloat16
    fp8 = mybir.dt.float8e4
    AF = mybir.ActivationFunctionType
    AX = mybir.AxisListType
    ALU = mybir.AluOpType
    DR = mybir.MatmulPerfMode.DoubleRow

    # Pin every activation to one LUT set so the table-load pass emits a
    # single load instead of thrashing between per-function default sets.
    orig_tables = bacc.get_activation_tables

    def pinned_tables(arch):
        return {
            name: (funcs if name == ACT_SET else set())
            for name, funcs in orig_tables(arch).items()
        }

    bacc.get_activation_tables = pinned_tables
    try:
        nc = bacc.Bacc(
            "TRN2",
            target_bir_lowering=False,
            debug=False,
            num_devices=NCORES,
        )
        xT = nc.declare_dram_parameter("xT", [D, W], bf16, isOutput=False)
        eye_d = nc.declare_dram_parameter("eye", [P, P], bf16, isOutput=False)
        negI_d = nc.declare_dram_parameter("negI", [P, P], bf16, isOutput=False)
        # single packed output: [rs (8*NG) | cs (248) | epos (8)]
        out_d = nc.declare_dram_parameter(
            "out", [P, 8 * NG + 8 * 31 + 8], f32, isOutput=True
        )
        rt_dram = nc.dram_tensor("rt_dram", [W], bf16)

        with tile.TileContext(nc) as tc:
            with (
                tc.tile_pool(name="big", bufs=1) as big,
                tc.tile_pool(name="ftp", bufs=2) as ftp,
                tc.tile_pool(name="work", bufs=3) as work,
                tc.tile_pool(name="nst", bufs=2) as nst,
                tc.tile_pool(name="psB", bufs=2, space="PSUM") as psB,
                tc.tile_pool(name="psS", bufs=1, space="PSUM") as psS,
            ):
                eye_sb = big.tile([P, P], bf16, tag="eye")
                negI_sb = big.tile([P, P], bf16, tag="negI")
                ones_sb = big.tile([P, 1], bf16, tag="ones")
                nc.vector.memset(ones_sb[:], 1.0)

                fqn = big.tile([P, KS, W], fp8, tag="fqn")
                R = big.tile([P, W], bf16, tag="R")
                rbf = big.tile([P, TCOL], bf16, tag="rbf")
                out_sb = big.tile([P, 8 * NG + 8 * 31 + 8], f32, tag="osb")
                rs_sb = out_sb[:, 0 : 8 * NG]
                cs_sb = out_sb[:, 8 * NG : 8 * NG + 8 * 31]
                epos_sb = out_sb[:, 8 * NG + 8 * 31 : 8 * NG + 8 * 31 + 8]

                # PSUM accumulation groups are tracked at 2KB-bank (zero
                # region) granularity: pad both tiles to a full bank so no
                # two concurrently-open groups ever share a bank.
                ss = psS.tile(
                    [P, TCOL], f32, tag="ss", name="ss", padded_shape=[P, 512]
                )
                cs_ps = psS.tile(
                    [P, 8 * 31], f32, tag="csps", name="csps", padded_shape=[P, 512]
                )

                fts = {}
                nstages = {}

                def prep_load(g):
                    tb0, tb1 = GBOUNDS[g], GBOUNDS[g + 1]
                    c0, w = tb0 * P, (tb1 - tb0) * P
                    for ks in range(KS):
                        ft = ftp.tile(
                            [P, w],
                            bf16,
                            tag=f"fT{ks}",
                            name=f"fT{ks}",
                            bufs=2,
                            padded_shape=[P, GW],
                        )
                        fts[(ks, g)] = ft
                        nc.sync.dma_start(
                            ft[:], xT[ks * P : (ks + 1) * P, c0 : c0 + w]
                        )

                def prep_a(g):
                    tb0, tb1 = GBOUNDS[g], GBOUNDS[g + 1]
                    nt = tb1 - tb0
                    c0, w = tb0 * P, nt * P
                    # All of ss lives in one PSUM bank, and group state is
                    # bank-granular — so run each column's 8-matmul k-accum
                    # group to completion (ks-inner) before opening the next.
                    sqs = {}
                    for ks in range(KS):
                        sq = sqs[ks] = work.tile(
                            [P, w],
                            bf16,
                            tag=f"sq{ks}",
                            name="sq",
                            bufs=2,
                            padded_shape=[P, GW],
                        )
                        nc.vector.tensor_mul(sq[:], fts[(ks, g)][:], fts[(ks, g)][:])
                    for s in range(nt):
                        t = tb0 + s
                        for ks in range(KS):
                            nc.tensor.matmul(
                                ss[:, t : t + 1],
                                sqs[ks][:, s * P : (s + 1) * P],
                                ones_sb[:],
                                start=(ks == 0),
                                stop=(ks == KS - 1),
                            )
                    # rsqrt with the fp8 scale folded in: SCALE * ss^-0.5
                    lnss = work.tile(
                        [P, nt], f32, tag="lnss", name="lnss", padded_shape=[P, 8]
                    )
                    nc.scalar.activation(
                        lnss[:],
                        ss[:, tb0:tb1],
                        AF.Ln,
                        scale=1.0 / (SCALE * SCALE),
                    )
                    nc.scalar.activation(
                        rbf[:, tb0:tb1], lnss[:], AF.Exp, scale=-0.5
                    )

                def prep_r(g):
                    tb0, tb1 = GBOUNDS[g], GBOUNDS[g + 1]
                    c0, w = tb0 * P, (tb1 - tb0) * P
                    # transposed write to DRAM (rt[128t+p] = rbf[p,t]; DRAM
                    # strides are unconstrained), then stride-0 partition
                    # broadcast-read back into all 128 partitions of R.
                    # On the SWDGE ring: HWDGE rings would head-of-line block
                    # the issuing engine's SEQ (R waits on rt's completion).
                    nc.gpsimd.dma_start(
                        rt_dram[c0 : c0 + w].rearrange("(t p) -> p t", p=P),
                        rbf[:, tb0:tb1],
                    )
                    nc.gpsimd.dma_start(
                        R[:, c0 : c0 + w],
                        rt_dram[c0 : c0 + w]
                        .rearrange("(a x) -> a x", a=1)
                        .broadcast_to((P, w)),
                    )

                def prep_b(g):
                    tb0, tb1 = GBOUNDS[g], GBOUNDS[g + 1]
                    c0, w = tb0 * P, (tb1 - tb0) * P
                    nstage = nst.tile(
                        [P, KS, w],
                        bf16,
                        tag="nst",
                        name="nst",
                        padded_shape=[P, KS, GW],
                    )
                    nstages[g] = nstage
                    for ks in range(KS):
                        nc.vector.tensor_mul(
                            nstage[:, ks, :], fts[(ks, g)][:], R[:, c0 : c0 + w]
                        )
                    nc.gpsimd.dma_start(fqn[:, :, c0 : c0 + w], nstage[:, :, :])

                def main_chunk(m, g):
                    # group-aligned chunk: intersection of row-tile m's
                    # offset strip (tiles m..m+32) with column group g —
                    # ready as soon as cast(g) lands
                    t0 = max(GBOUNDS[g], m)
                    t1 = min(GBOUNDS[g + 1], m + NDOFF)
                    if t1 <= t0:
                        return
                    wch = (t1 - t0) * P
                    col0 = t0 * P
                    B = psB.tile(
                        [P, wch], f32, tag="B", name="B", padded_shape=[P, GW]
                    )
                    nh = (wch + 511) // 512
                    for j in range(4):
                        for h in range(nh):
                            h0 = h * 512
                            h1 = min(wch, h0 + 512)
                            nc.tensor.matmul(
                                B[:, h0:h1],
                                fqn[:, 2 * j : 2 * j + 2, m * P : (m + 1) * P],
                                fqn[:, 2 * j : 2 * j + 2, col0 + h0 : col0 + h1],
                                start=(j == 0),
                                stop=(j == 3),
                                perf_mode=DR,
                            )
                        if j == 2 and t0 == m:
                            # self-block diag mask: B[:, :128] += negI^T @ I
                            nc.tensor.matmul(
                                B[:, 0:P],
                                negI_sb[:],
                                eye_sb[:],
                                start=False,
                                stop=False,
                            )
                    E = work.tile(
                        [P, wch],
                        bf16,
                        tag="E",
                        name="E",
                        bufs=6,
                        padded_shape=[P, GW],
                    )
                    nc.scalar.activation(
                        E[:],
                        B[:],
                        AF.Exp,
                        scale=S_EXP,
                        accum_out=rs_sb[:, NG * m + g : NG * m + g + 1],
                    )
                    for t in range(t0, t1):
                        d = t - m
                        if d == 0 or d == 32:
                            continue  # self block / d=32: row side only
                        s0 = (t - t0) * P
                        nc.tensor.matmul(
                            cs_ps[:, m * 31 + d - 1 : m * 31 + d],
                            E[:, s0 : s0 + P],
                            ones_sb[:],
                            start=True,
                            stop=True,
                        )
                    if t0 <= m + 32 < t1:
                        # positive pair: diag of the exp'd d=32 tile
                        s0 = (m + 32 - t0) * P
                        pd = work.tile([P, P], bf16, tag="pd", name="pd")
                        nc.vector.tensor_mul(pd[:], E[:, s0 : s0 + P], eye_sb[:])
                        nc.vector.reduce_sum(
                            epos_sb[:, m : m + 1], pd[:], axis=AX.X
                        )

                # Emission order: A(g) = loads/squares/sumsq/rsqrt,
                # R(g) = scale transpose/broadcast, B(g) = normalize+cast.
                # A(g+1) is emitted before B(g) so DVE has square work while
                # R(g) settles; each cast(g) unlocks the 8 (m, g) chunks.
                prep_load(0)
                prep_a(0)
                # eye/negI aren't needed until the first diag-mask matmul
                # (~25us in): keep the SP ring clear for the g0/g1 loads
                nc.sync.dma_start(eye_sb[:], eye_d[:])
                nc.sync.dma_start(negI_sb[:], negI_d[:])
                prep_load(1)
                prep_a(1)
                prep_r(0)
                for g in range(NG):
                    # rt/R of g+1 ahead of cast(g) on the Pool queue: its
                    # rsqrt fired a DVE-period ago, so the hop completes
                    # while cast(g) still waits on norm(g)
                    if g + 1 < NG:
                        prep_r(g + 1)
                    prep_b(g)
                    if g + 2 < NG:
                        prep_load(g + 2)
                        prep_a(g + 2)
                    for m in range(8):
                        main_chunk(m, g)

                nc.vector.tensor_copy(cs_sb, cs_ps[:])
                nc.sync.dma_start(out_d[:], out_sb[:])

        nc.compile()
    finally:
        bacc.get_activation_tables = orig_tables
    return nc


def _get_program():
    if "nc" not in _cache:
        _cache["nc"] = _build_program()
    return _cache["nc"]


def kernel(features: np.ndarray, _trace: bool = False):
    import ml_dtypes
    from concourse.bass_utils import run_bass_kernel_spmd

    nc = _get_program()
    features = np.ascontiguousarray(features, dtype=np.float32)
    eye = np.eye(P, dtype=ml_dtypes.bfloat16)
    negI = (DIAG_NEG * np.eye(P)).astype(ml_dtypes.bfloat16)
    in_maps = []
    for k in range(NCORES):
        xk = np.roll(features, -N // NCORES * k, axis=0)
        xT = np.ascontiguousarray(xk.astype(ml_dtypes.bfloat16).T[:, :W])
        in_maps.append({"xT": xT, "eye": eye, "negI": negI})
    res = run_bass_kernel_spmd(
        nc,
        in_maps,
        core_ids=list(range(NCORES)),
        trace=_trace,
    )

    acc = np.zeros((64, P), np.float64)
    pos_ln = np.zeros((64, P), np.float64)
    for k in range(NCORES):
        o = res.results[k]["out"].astype(np.float64)
        rs = o[:, 0 : 8 * NG]
        cs = o[:, 8 * NG : 8 * NG + 248]
        epos = o[:, 8 * NG + 248 :]
        for m in range(8):
            t = (m + 8 * k) % 64
            acc[t] += rs[:, NG * m : NG * (m + 1)].sum(axis=1)
            pos_ln[t] = np.log(epos[:, m])
            for d in range(1, 32):
                acc[(m + d + 8 * k) % 64] += cs[:, m * 31 + d - 1]
    losses = np.log(acc) - pos_ln
    loss = np.float32(losses.mean())
    if _trace:
        return loss, res
    return loss
